# revision 20
# baseline (speedup 1.0000x reference)
"""Trainium2 Bass kernel for nn_MultiHeadSelfAttention_24541443130010.

Full inputs in, full output out. Sharding: DP over batch (4) x TP over heads (2)
across 8 NeuronCores. Core c handles batch c//2 with q-heads [8r, 8r+8) and
kv-heads [2r, 2r+2), r = c%2. Wo is row-sliced per rank; the TP partial sums
are reduced on the host after gather.

Device kernel (per core, fp32 storage / float32r matmuls):
  qkv^T projection from host-pretransposed x^T, RoPE with the head-dim
  de-interleave permutation folded into the Wq/Wk columns (rotate_half becomes
  a 64-partition block swap), flash-style attention computing S^T = K Q^T tiles
  (softmax denominators via one-hot ones-matmuls on PE; no max subtraction --
  scores are O(1) here), causal block skipping, then the output projection.
"""
import os
import numpy as np
from contextlib import ExitStack

import concourse.bass as bass
import concourse.mybir as mybir
import concourse.tile as tile
from concourse import bass_utils, bacc

f32 = mybir.dt.float32
f32r = mybir.dt.float32r
AF = mybir.ActivationFunctionType

B = 4
S = 2048
D = 2048
DH = 128
HQ = 16
HKV = 4
TP = 2
HL = HQ // TP       # 8 local q heads
KVL = HKV // TP     # 2 local kv heads
CH = 512            # q-chunk size
NCH = S // CH       # 4
NKT = S // DH       # 16 k-tiles
NDC = D // DH       # 16 contraction chunks
SCALE = 1.0 / float(np.sqrt(DH))

LAST_EXEC_NS = None
_NC_CACHE = None


def build_nc():
    nc = bacc.Bacc("TRN2", target_bir_lowering=False, debug=False)
    xt_d = nc.dram_tensor("xt", [D, S], f32r, kind="ExternalInput").ap()
    wqk_d = nc.dram_tensor("wqk", [HL + KVL, 2, DH, 8, DH], f32r, kind="ExternalInput").ap()
    wv_d = nc.dram_tensor("wv", [D, KVL * DH], f32r, kind="ExternalInput").ap()
    wo_d = nc.dram_tensor("wo", [HL * DH, D], f32r, kind="ExternalInput").ap()
    cos_d = nc.dram_tensor("cost", [DH, S], f32r, kind="ExternalInput").ap()
    sin_d = nc.dram_tensor("sint", [DH, S], f32r, kind="ExternalInput").ap()
    msk_d = nc.dram_tensor("dmask", [DH, 4, CH], f32r, kind="ExternalInput").ap()
    oh_d = nc.dram_tensor("onehot", [DH, 4, 4], f32r, kind="ExternalInput").ap()
    y_d = nc.dram_tensor("y", [S, D], f32, kind="ExternalOutput").ap()

    wo_r = wo_d.rearrange("(h p) o -> p h o", p=DH)       # [128, 8, 2048]

    with tile.TileContext(nc) as tc, ExitStack() as ctx:
        def pool(name, bufs, space="SBUF"):
            return ctx.enter_context(tc.tile_pool(name=name, bufs=bufs, space=space))

        xt_p = pool("xt", 6)
        wqk_p = pool("wqk", 4)
        per_p = pool("per", 1)       # persistent singles (distinct tags)
        qT_p = pool("qT", 1)
        oT_p = pool("oT", 1)
        pT_p = pool("pT", 5)
        wo_p = pool("wo", 3)
        y_p = pool("y", 2)
        rl_p = pool("rl", 1)
        rlb_p = pool("rlb", 1)
        sw_p = pool("sw", 2)
        tab_p = pool("tab", 1)
        psG = pool("psG", 3, "PSUM")
        psO = pool("psO", 4, "PSUM")
        psL = pool("psL", 1, "PSUM")

        # persistent SBUF residents
        wv_sb = per_p.tile([DH, NDC, KVL * DH], f32r, tag="wv")
        kT_sb = per_p.tile([DH, KVL, S], f32r, tag="kT")
        v_sb = per_p.tile([DH, NKT, KVL * DH], f32r, tag="v")
        msk_sb = per_p.tile([DH, 4, CH], f32r, tag="msk")
        oh_sb = per_p.tile([DH, 4, 4], f32r, tag="oh")

        def load_persistents():
            wv_r = wv_d.rearrange("(dc p) v -> p dc v", p=DH)
            nc.sync.dma_start(wv_sb[:], wv_r[:])
            nc.sync.dma_start(msk_sb[:], msk_d[:])
            nc.sync.dma_start(oh_sb[:], oh_d[:])

        def rope(tgt, cos_c, sin_c):
            """In-place RoPE on a [128, CH] slice (permuted layout:
            rotate_half = 64-partition block swap)."""
            sw = sw_p.tile([DH, CH], f32r, tag="sw", name="sw")
            nc.gpsimd.dma_start(sw[0:64, :], tgt[64:128, :])
            nc.gpsimd.dma_start(sw[64:128, :], tgt[0:64, :])
            nc.vector.tensor_mul(tgt, tgt, cos_c[:])
            nc.vector.tensor_mul(sw[:], sw[:], sin_c[:])
            nc.vector.tensor_add(tgt, tgt, sw[:])

        def load_and_project(qc):
            """x^T chunk load + Q/K projection (with inline RoPE) + V proj."""
            s0 = qc * CH
            xt_g = [xt_p.tile([DH, 4, CH], f32r, tag="xt", name=f"xt_g{_i}")
                    for _i in range(4)]
            for Dc in range(NDC):
                nc.sync.dma_start(xt_g[Dc // 4][:, Dc % 4, :],
                                  xt_d[Dc * DH:(Dc + 1) * DH, s0:s0 + CH])

            def xt_slice(Dc, lo=0, hi=CH):
                return xt_g[Dc // 4][:, Dc % 4, lo:hi]

            cos_c = tab_p.tile([DH, CH], f32r, tag="cos", name="cos_c")
            sin_c = tab_p.tile([DH, CH], f32r, tag="sin", name="sin_c")
            nc.sync.dma_start(cos_c[:], cos_d[:, s0:s0 + CH])
            nc.sync.dma_start(sin_c[:], sin_d[:, s0:s0 + CH])
            qT_sb = qT_p.tile([DH, HL, CH], f32r, tag="qT", name="qT_sb")
            for f in range(HL + KVL):
                ps = psG.tile([DH, CH], f32, tag="g")
                for dcg in range(2):
                    wt = wqk_p.tile([DH, 8, DH], f32r, tag="w")
                    nc.sync.dma_start(wt[:], wqk_d[f, dcg])
                    for dci in range(8):
                        Dc = dcg * 8 + dci
                        nc.tensor.matmul(ps[:], wt[:, dci, :], xt_slice(Dc),
                                         start=(Dc == 0), stop=(Dc == NDC - 1))
                dst = qT_sb[:, f, :] if f < HL else kT_sb[:, f - HL, s0:s0 + CH]
                nc.vector.tensor_copy(dst, ps[:])
                rope(dst, cos_c, sin_c)
                if qc == 0 and f == 0:
                    load_persistents()

            for st in range(4):
                kt_g = qc * 4 + st
                psv = psG.tile([DH, KVL * DH], f32, tag="g")
                for Dc in range(NDC):
                    nc.tensor.matmul(psv[:], xt_slice(Dc, st * DH, (st + 1) * DH),
                                     wv_sb[:, Dc, :], start=(Dc == 0), stop=(Dc == NDC - 1))
                nc.vector.tensor_copy(v_sb[:, kt_g, :], psv[:])
            return qT_sb

        qT_sb = load_and_project(0)
        for qc in range(NCH):
            s0 = qc * CH
            # ---- attention for this q-chunk ----
            oT_sb = oT_p.tile([DH, HL, CH], f32r, tag="oT")
            nkt = (qc + 1) * 4
            for g in range(2):
                po = [psO.tile([DH, CH], f32, tag="o", name=f"po{_i}") for _i in range(4)]
                pl = psL.tile([DH, CH], f32, tag="l")
                for kt in range(nkt):
                    pts = []
                    for hi in range(4):
                        h = g * 4 + hi
                        ps = psG.tile([DH, CH], f32, tag="g")
                        nc.tensor.matmul(ps[:], kT_sb[:, g, kt * DH:(kt + 1) * DH],
                                         qT_sb[:, h, :], start=True, stop=True)
                        pt = pT_p.tile([DH, CH], f32r, tag="p")
                        nc.scalar.activation(pt[:], ps[:], AF.Exp, scale=SCALE)
                        if kt >= qc * 4:
                            r = kt - qc * 4
                            nc.vector.tensor_mul(pt[:], pt[:], msk_sb[:, r, :])
                        pts.append(pt)
                    for hi in range(4):
                        nc.tensor.matmul(po[hi][:], v_sb[:, kt, g * DH:(g + 1) * DH],
                                         pts[hi][:], start=(kt == 0), stop=(kt == nkt - 1))
                    for hi in range(4):
                        nc.tensor.matmul(pl[0:4, :], oh_sb[:, hi, :], pts[hi][:],
                                         start=(kt == 0 and hi == 0),
                                         stop=(kt == nkt - 1 and hi == 3))
                # reciprocal first (frees the l-bank for the next group),
                # then drain the O accumulators (unnormalized)
                rl = rl_p.tile([4, CH], f32, tag="rl")
                nc.vector.reciprocal(rl[:], pl[0:4, :])
                for hi in range(4):
                    nc.vector.tensor_copy(oT_sb[:, g * 4 + hi, :], po[hi][:])
                for hi in range(4):
                    rlh = rl_p.tile([1, CH], f32, tag="rlh", name="rlh")
                    nc.gpsimd.dma_start(rlh[0:1, :], rl[hi:hi + 1, :])
                    rlb = rlb_p.tile([DH, CH], f32, tag="rlb")
                    nc.gpsimd.partition_broadcast(rlb[:], rlh[0:1, :])
                    nc.vector.tensor_mul(oT_sb[:, g * 4 + hi, :],
                                         oT_sb[:, g * 4 + hi, :], rlb[:])

            # ---- next chunk's projection (fills PE while the softmax
            # normalization round-trip completes) ----
            if qc + 1 < NCH:
                qT_next = load_and_project(qc + 1)
            else:
                qT_next = None

            # ---- output projection ----
            for oc in range(4):
                wta = wo_p.tile([DH, 4, CH], f32r, tag="wo", name="wta")
                wtb = wo_p.tile([DH, 4, CH], f32r, tag="wo", name="wtb")
                nc.sync.dma_start(wta[:], wo_r[:, 0:4, oc * CH:(oc + 1) * CH])
                nc.sync.dma_start(wtb[:], wo_r[:, 4:8, oc * CH:(oc + 1) * CH])
                for qt in range(4):
                    psy = psG.tile([DH, CH], f32, tag="g")
                    for hid in range(HL):
                        wt_h = wta[:, hid, :] if hid < 4 else wtb[:, hid - 4, :]
                        nc.tensor.matmul(psy[:], oT_sb[:, hid, qt * DH:(qt + 1) * DH],
                                         wt_h, start=(hid == 0), stop=(hid == HL - 1))
                    yt = y_p.tile([DH, CH], f32, tag="y")
                    nc.vector.tensor_copy(yt[:], psy[:])
                    nc.sync.dma_start(
                        y_d[s0 + qt * DH: s0 + (qt + 1) * DH, oc * CH:(oc + 1) * CH], yt[:])
            qT_sb = qT_next

    nc.compile()
    return nc


def _get_nc():
    global _NC_CACHE
    if _NC_CACHE is None:
        _NC_CACHE = build_nc()
    return _NC_CACHE


def _host_prep(x, Wqkv, Wo):
    perm = np.concatenate([np.arange(0, DH, 2), np.arange(1, DH, 2)])
    # RoPE tables, matching reference numerics (float32 throughout)
    theta = (1.0 / (np.float32(10000.0) **
                    (np.arange(0, DH, 2, dtype=np.float32) / np.float32(DH))))
    theta = np.repeat(theta.astype(np.float32), 2)                  # [128]
    ang = (np.arange(S, dtype=np.float32)[:, None] * theta[None, :])  # [S, 128]
    cosT = np.ascontiguousarray(np.cos(ang).astype(np.float32).T[perm, :])  # [128, S]
    sinT = np.ascontiguousarray(np.sin(ang).astype(np.float32).T[perm, :])
    sinT[:64, :] *= np.float32(-1.0)

    kk = np.arange(DH)[:, None]
    qq = np.arange(CH)[None, :]
    dmask = np.zeros((DH, 4, CH), np.float32)
    for r in range(4):
        dmask[:, r, :] = (qq >= r * DH + kk).astype(np.float32)
    oh = np.zeros((DH, 4, 4), np.float32)
    for hi in range(4):
        oh[:, hi, hi] = 1.0

    in_maps = []
    for c in range(8):
        b, r = c // 2, c % 2
        xt = np.ascontiguousarray(x[b].T)
        cols = []
        for h in range(HL):
            base = (r * HL + h) * DH
            cols.append(Wqkv[:, base + perm])
        for l in range(KVL):
            base = HQ * DH + (r * KVL + l) * DH
            cols.append(Wqkv[:, base + perm])
        wqk = np.concatenate(cols, axis=1)              # [2048, 1280]
        # pre-tile: [f, dcg, p, dc, fi] with 4KB-contiguous per-partition lines
        wqk = np.ascontiguousarray(
            wqk.reshape(2, 8, DH, HL + KVL, DH)          # [dcg, dc, p, f, fi]
               .transpose(3, 0, 2, 1, 4))                # [f, dcg, p, dc, fi]
        vbase = (HQ + HKV) * DH
        wv = np.ascontiguousarray(Wqkv[:, vbase + r * KVL * DH: vbase + (r + 1) * KVL * DH])
        wo = np.ascontiguousarray(Wo[r * HL * DH:(r + 1) * HL * DH, :])
        in_maps.append(dict(xt=xt, wqk=wqk, wv=wv, wo=wo, cost=cosT, sint=sinT,
                            dmask=dmask, onehot=oh))
    return in_maps


def kernel(x, Wqkv, Wo, mask):
    global LAST_EXEC_NS
    x = np.asarray(x, dtype=np.float32)
    Wqkv = np.asarray(Wqkv, dtype=np.float32)
    Wo = np.asarray(Wo, dtype=np.float32)
    in_maps = _host_prep(x, Wqkv, Wo)
    nc = _get_nc()
    trace = os.environ.get("KTRACE", "0") == "1"
    res = bass_utils.run_bass_kernel_spmd(nc, in_maps, core_ids=list(range(8)), trace=trace)
    LAST_EXEC_NS = res.exec_time_ns
    out = np.empty((B, S, D), np.float32)
    for b in range(B):
        out[b] = res.results[2 * b]["y"] + res.results[2 * b + 1]["y"]
    return out


# revision 23
# speedup vs baseline: 1.0167x; 1.0167x over previous
"""Trainium2 Bass kernel for nn_MultiHeadSelfAttention_24541443130010.

Full inputs in, full output out. Sharding: DP over batch (4) x TP over heads (2)
across 8 NeuronCores. Core c handles batch c//2 with q-heads [8r, 8r+8) and
kv-heads [2r, 2r+2), r = c%2. Wo is row-sliced per rank; the TP partial sums
are reduced on the host after gather.

Device kernel (per core, fp32 storage / float32r matmuls):
  qkv^T projection from host-pretransposed x^T, RoPE with the head-dim
  de-interleave permutation folded into the Wq/Wk columns (rotate_half becomes
  a 64-partition block swap), flash-style attention computing S^T = K Q^T tiles
  (softmax denominators via one-hot ones-matmuls on PE; no max subtraction --
  scores are O(1) here), causal block skipping, then the output projection.
"""
import os
import numpy as np
from contextlib import ExitStack

import concourse.bass as bass
import concourse.mybir as mybir
import concourse.tile as tile
from concourse import bass_utils, bacc

f32 = mybir.dt.float32
f32r = mybir.dt.float32r
AF = mybir.ActivationFunctionType

B = 4
S = 2048
D = 2048
DH = 128
HQ = 16
HKV = 4
TP = 2
HL = HQ // TP       # 8 local q heads
KVL = HKV // TP     # 2 local kv heads
CH = 512            # q-chunk size
NCH = S // CH       # 4
NKT = S // DH       # 16 k-tiles
NDC = D // DH       # 16 contraction chunks
SCALE = 1.0 / float(np.sqrt(DH))

LAST_EXEC_NS = None
_NC_CACHE = None


def build_nc():
    nc = bacc.Bacc("TRN2", target_bir_lowering=False, debug=False)
    xt_d = nc.dram_tensor("xt", [D, S], f32r, kind="ExternalInput").ap()
    wqk_d = nc.dram_tensor("wqk", [HL + KVL, 2, DH, 8, DH], f32r, kind="ExternalInput").ap()
    wv_d = nc.dram_tensor("wv", [D, KVL * DH], f32r, kind="ExternalInput").ap()
    wo_d = nc.dram_tensor("wo", [HL * DH, D], f32r, kind="ExternalInput").ap()
    cos_d = nc.dram_tensor("cost", [DH, S], f32r, kind="ExternalInput").ap()
    sin_d = nc.dram_tensor("sint", [DH, S], f32r, kind="ExternalInput").ap()
    msk_d = nc.dram_tensor("dmask", [DH, 4, CH], f32r, kind="ExternalInput").ap()
    oh_d = nc.dram_tensor("onehot", [DH, 4, 4], f32r, kind="ExternalInput").ap()
    y_d = nc.dram_tensor("y", [S, D], f32, kind="ExternalOutput").ap()

    wo_r = wo_d.rearrange("(h p) o -> p h o", p=DH)       # [128, 8, 2048]

    with tile.TileContext(nc) as tc, ExitStack() as ctx:
        def pool(name, bufs, space="SBUF"):
            return ctx.enter_context(tc.tile_pool(name=name, bufs=bufs, space=space))

        xt_p = pool("xt", 6)
        wqk_p = pool("wqk", 4)
        per_p = pool("per", 1)       # persistent singles (distinct tags)
        qT_p = pool("qT", 1)
        oT_p = pool("oT", 1)
        pT_p = pool("pT", 6)
        wo_p = pool("wo", 3)
        y_p = pool("y", 2)
        rl_p = pool("rl", 1)
        rlb_p = pool("rlb", 1)
        sw_p = pool("sw", 2)
        tab_p = pool("tab", 1)
        psG = pool("psG", 3, "PSUM")
        psO = pool("psO", 4, "PSUM")
        psL = pool("psL", 1, "PSUM")

        # persistent SBUF residents
        wv_sb = per_p.tile([DH, NDC, KVL * DH], f32r, tag="wv")
        kT_sb = per_p.tile([DH, KVL, S], f32r, tag="kT")
        v_sb = per_p.tile([DH, NKT, KVL * DH], f32r, tag="v")
        msk_sb = per_p.tile([DH, 4, CH], f32r, tag="msk")
        oh_sb = per_p.tile([DH, 4, 4], f32r, tag="oh")

        def load_persistents():
            wv_r = wv_d.rearrange("(dc p) v -> p dc v", p=DH)
            nc.sync.dma_start(wv_sb[:], wv_r[:])
            nc.sync.dma_start(msk_sb[:], msk_d[:])
            nc.sync.dma_start(oh_sb[:], oh_d[:])

        def rope(tgt, cos_c, sin_c):
            """In-place RoPE on a [128, CH] slice (permuted layout:
            rotate_half = 64-partition block swap)."""
            sw = sw_p.tile([DH, CH], f32r, tag="sw", name="sw")
            nc.gpsimd.dma_start(sw[0:64, :], tgt[64:128, :])
            nc.gpsimd.dma_start(sw[64:128, :], tgt[0:64, :])
            nc.vector.tensor_mul(tgt, tgt, cos_c[:])
            nc.vector.tensor_mul(sw[:], sw[:], sin_c[:])
            nc.vector.tensor_add(tgt, tgt, sw[:])

        def load_and_project(qc):
            """x^T chunk load + Q/K projection (with inline RoPE) + V proj."""
            s0 = qc * CH
            xt_g = [xt_p.tile([DH, 4, CH], f32r, tag="xt", name=f"xt_g{_i}")
                    for _i in range(4)]
            for Dc in range(NDC):
                nc.sync.dma_start(xt_g[Dc // 4][:, Dc % 4, :],
                                  xt_d[Dc * DH:(Dc + 1) * DH, s0:s0 + CH])

            def xt_slice(Dc, lo=0, hi=CH):
                return xt_g[Dc // 4][:, Dc % 4, lo:hi]

            def v_proj():
                # V PSUM drains land on an empty DVE queue (before the RoPE
                # backlog), so banks recycle fast.
                for st in range(4):
                    kt_g = qc * 4 + st
                    psv = psG.tile([DH, KVL * DH], f32, tag="g")
                    for Dc in range(NDC):
                        nc.tensor.matmul(psv[:], xt_slice(Dc, st * DH, (st + 1) * DH),
                                         wv_sb[:, Dc, :],
                                         start=(Dc == 0), stop=(Dc == NDC - 1))
                    nc.vector.tensor_copy(v_sb[:, kt_g, :], psv[:])

            if qc > 0:
                v_proj()
            cos_c = tab_p.tile([DH, CH], f32r, tag="cos", name="cos_c")
            sin_c = tab_p.tile([DH, CH], f32r, tag="sin", name="sin_c")
            nc.sync.dma_start(cos_c[:], cos_d[:, s0:s0 + CH])
            nc.sync.dma_start(sin_c[:], sin_d[:, s0:s0 + CH])
            qT_sb = qT_p.tile([DH, HL, CH], f32r, tag="qT", name="qT_sb")
            # K heads first: chunk-0 attention (all-diagonal) needs roped k^T
            # before anything else.
            for f in [HL, HL + 1] + list(range(HL)):
                ps = psG.tile([DH, CH], f32, tag="g")
                for dcg in range(2):
                    wt = wqk_p.tile([DH, 8, DH], f32r, tag="w")
                    nc.sync.dma_start(wt[:], wqk_d[f, dcg])
                    for dci in range(8):
                        Dc = dcg * 8 + dci
                        nc.tensor.matmul(ps[:], wt[:, dci, :], xt_slice(Dc),
                                         start=(Dc == 0), stop=(Dc == NDC - 1))
                dst = qT_sb[:, f, :] if f < HL else kT_sb[:, f - HL, s0:s0 + CH]
                nc.vector.tensor_copy(dst, ps[:])
                rope(dst, cos_c, sin_c)
                if qc == 0 and f == HL + 1:
                    load_persistents()
            if qc == 0:
                v_proj()

            return qT_sb

        qT_sb = load_and_project(0)
        for qc in range(NCH):
            s0 = qc * CH
            # ---- attention for this q-chunk ----
            oT_sb = oT_p.tile([DH, HL, CH], f32r, tag="oT")
            nkt = (qc + 1) * 4
            for g in range(2):
                po = [psO.tile([DH, CH], f32, tag="o", name=f"po{_i}") for _i in range(4)]
                pl = psL.tile([DH, CH], f32, tag="l")
                for kt in range(nkt):
                    pts = []
                    for hi in range(4):
                        h = g * 4 + hi
                        ps = psG.tile([DH, CH], f32, tag="g")
                        nc.tensor.matmul(ps[:], kT_sb[:, g, kt * DH:(kt + 1) * DH],
                                         qT_sb[:, h, :], start=True, stop=True)
                        pt = pT_p.tile([DH, CH], f32r, tag="p")
                        nc.scalar.activation(pt[:], ps[:], AF.Exp, scale=SCALE)
                        if kt >= qc * 4:
                            r = kt - qc * 4
                            nc.vector.tensor_mul(pt[:], pt[:], msk_sb[:, r, :])
                        pts.append(pt)
                    for hi in range(4):
                        nc.tensor.matmul(po[hi][:], v_sb[:, kt, g * DH:(g + 1) * DH],
                                         pts[hi][:], start=(kt == 0), stop=(kt == nkt - 1))
                    for hi in range(4):
                        nc.tensor.matmul(pl[0:4, :], oh_sb[:, hi, :], pts[hi][:],
                                         start=(kt == 0 and hi == 0),
                                         stop=(kt == nkt - 1 and hi == 3))
                # reciprocal first (frees the l-bank for the next group),
                # then drain the O accumulators (unnormalized)
                rl = rl_p.tile([4, CH], f32, tag="rl")
                nc.vector.reciprocal(rl[:], pl[0:4, :])
                for hi in range(4):
                    nc.vector.tensor_copy(oT_sb[:, g * 4 + hi, :], po[hi][:])
                for hi in range(4):
                    rlh = rl_p.tile([1, CH], f32, tag="rlh", name="rlh")
                    nc.gpsimd.dma_start(rlh[0:1, :], rl[hi:hi + 1, :])
                    rlb = rlb_p.tile([DH, CH], f32, tag="rlb")
                    nc.gpsimd.partition_broadcast(rlb[:], rlh[0:1, :])
                    nc.vector.tensor_mul(oT_sb[:, g * 4 + hi, :],
                                         oT_sb[:, g * 4 + hi, :], rlb[:])

            # ---- next chunk's projection (fills PE while the softmax
            # normalization round-trip completes) ----
            if qc + 1 < NCH:
                qT_next = load_and_project(qc + 1)
            else:
                qT_next = None

            # ---- output projection ----
            for oc in range(4):
                wta = wo_p.tile([DH, 4, CH], f32r, tag="wo", name="wta")
                wtb = wo_p.tile([DH, 4, CH], f32r, tag="wo", name="wtb")
                nc.sync.dma_start(wta[:], wo_r[:, 0:4, oc * CH:(oc + 1) * CH])
                nc.sync.dma_start(wtb[:], wo_r[:, 4:8, oc * CH:(oc + 1) * CH])
                for qt in range(4):
                    psy = psG.tile([DH, CH], f32, tag="g")
                    for hid in range(HL):
                        wt_h = wta[:, hid, :] if hid < 4 else wtb[:, hid - 4, :]
                        nc.tensor.matmul(psy[:], oT_sb[:, hid, qt * DH:(qt + 1) * DH],
                                         wt_h, start=(hid == 0), stop=(hid == HL - 1))
                    yt = y_p.tile([DH, CH], f32, tag="y")
                    nc.vector.tensor_copy(yt[:], psy[:])
                    nc.sync.dma_start(
                        y_d[s0 + qt * DH: s0 + (qt + 1) * DH, oc * CH:(oc + 1) * CH], yt[:])
            qT_sb = qT_next

    nc.compile()
    return nc


def _get_nc():
    global _NC_CACHE
    if _NC_CACHE is None:
        _NC_CACHE = build_nc()
    return _NC_CACHE


def _host_prep(x, Wqkv, Wo):
    perm = np.concatenate([np.arange(0, DH, 2), np.arange(1, DH, 2)])
    # RoPE tables, matching reference numerics (float32 throughout)
    theta = (1.0 / (np.float32(10000.0) **
                    (np.arange(0, DH, 2, dtype=np.float32) / np.float32(DH))))
    theta = np.repeat(theta.astype(np.float32), 2)                  # [128]
    ang = (np.arange(S, dtype=np.float32)[:, None] * theta[None, :])  # [S, 128]
    cosT = np.ascontiguousarray(np.cos(ang).astype(np.float32).T[perm, :])  # [128, S]
    sinT = np.ascontiguousarray(np.sin(ang).astype(np.float32).T[perm, :])
    sinT[:64, :] *= np.float32(-1.0)

    kk = np.arange(DH)[:, None]
    qq = np.arange(CH)[None, :]
    dmask = np.zeros((DH, 4, CH), np.float32)
    for r in range(4):
        dmask[:, r, :] = (qq >= r * DH + kk).astype(np.float32)
    oh = np.zeros((DH, 4, 4), np.float32)
    for hi in range(4):
        oh[:, hi, hi] = 1.0

    in_maps = []
    for c in range(8):
        b, r = c // 2, c % 2
        xt = np.ascontiguousarray(x[b].T)
        cols = []
        for h in range(HL):
            base = (r * HL + h) * DH
            cols.append(Wqkv[:, base + perm])
        for l in range(KVL):
            base = HQ * DH + (r * KVL + l) * DH
            cols.append(Wqkv[:, base + perm])
        wqk = np.concatenate(cols, axis=1)              # [2048, 1280]
        # pre-tile: [f, dcg, p, dc, fi] with 4KB-contiguous per-partition lines
        wqk = np.ascontiguousarray(
            wqk.reshape(2, 8, DH, HL + KVL, DH)          # [dcg, dc, p, f, fi]
               .transpose(3, 0, 2, 1, 4))                # [f, dcg, p, dc, fi]
        vbase = (HQ + HKV) * DH
        wv = np.ascontiguousarray(Wqkv[:, vbase + r * KVL * DH: vbase + (r + 1) * KVL * DH])
        wo = np.ascontiguousarray(Wo[r * HL * DH:(r + 1) * HL * DH, :])
        in_maps.append(dict(xt=xt, wqk=wqk, wv=wv, wo=wo, cost=cosT, sint=sinT,
                            dmask=dmask, onehot=oh))
    return in_maps


def kernel(x, Wqkv, Wo, mask):
    global LAST_EXEC_NS
    x = np.asarray(x, dtype=np.float32)
    Wqkv = np.asarray(Wqkv, dtype=np.float32)
    Wo = np.asarray(Wo, dtype=np.float32)
    in_maps = _host_prep(x, Wqkv, Wo)
    nc = _get_nc()
    trace = os.environ.get("KTRACE", "0") == "1"
    res = bass_utils.run_bass_kernel_spmd(nc, in_maps, core_ids=list(range(8)), trace=trace)
    LAST_EXEC_NS = res.exec_time_ns
    out = np.empty((B, S, D), np.float32)
    for b in range(B):
        out[b] = res.results[2 * b]["y"] + res.results[2 * b + 1]["y"]
    return out


# revision 26
# speedup vs baseline: 1.0168x; 1.0001x over previous
"""Trainium2 Bass kernel for nn_MultiHeadSelfAttention_24541443130010.

Full inputs in, full output out. Sharding: DP over batch (4) x TP over heads (2)
across 8 NeuronCores. Core c handles batch c//2 with q-heads [8r, 8r+8) and
kv-heads [2r, 2r+2), r = c%2. Wo is row-sliced per rank; the TP partial sums
are reduced on the host after gather.

Device kernel (per core, fp32 storage / float32r matmuls):
  qkv^T projection from host-pretransposed x^T, RoPE with the head-dim
  de-interleave permutation folded into the Wq/Wk columns (rotate_half becomes
  a 64-partition block swap), flash-style attention computing S^T = K Q^T tiles
  (softmax denominators via one-hot ones-matmuls on PE; no max subtraction --
  scores are O(1) here), causal block skipping, then the output projection.
"""
import os
import numpy as np
from contextlib import ExitStack

import concourse.bass as bass
import concourse.mybir as mybir
import concourse.tile as tile
from concourse import bass_utils, bacc

f32 = mybir.dt.float32
f32r = mybir.dt.float32r
AF = mybir.ActivationFunctionType

B = 4
S = 2048
D = 2048
DH = 128
HQ = 16
HKV = 4
TP = 2
HL = HQ // TP       # 8 local q heads
KVL = HKV // TP     # 2 local kv heads
CH = 512            # q-chunk size
NCH = S // CH       # 4
NKT = S // DH       # 16 k-tiles
NDC = D // DH       # 16 contraction chunks
SCALE = 1.0 / float(np.sqrt(DH))

LAST_EXEC_NS = None
_NC_CACHE = None


def build_nc():
    nc = bacc.Bacc("TRN2", target_bir_lowering=False, debug=False)
    xt_d = nc.dram_tensor("xt", [D, S], f32r, kind="ExternalInput").ap()
    wqk_d = nc.dram_tensor("wqk", [HL + KVL, 2, DH, 8, DH], f32r, kind="ExternalInput").ap()
    wv_d = nc.dram_tensor("wv", [D, KVL * DH], f32r, kind="ExternalInput").ap()
    wo_d = nc.dram_tensor("wo", [HL * DH, D], f32r, kind="ExternalInput").ap()
    cos_d = nc.dram_tensor("cost", [DH, S], f32r, kind="ExternalInput").ap()
    sin_d = nc.dram_tensor("sint", [DH, S], f32r, kind="ExternalInput").ap()
    msk_d = nc.dram_tensor("dmask", [DH, 4, CH], f32r, kind="ExternalInput").ap()
    oh_d = nc.dram_tensor("onehot", [DH, 4, 4], f32r, kind="ExternalInput").ap()
    y_d = nc.dram_tensor("y", [S, D], f32, kind="ExternalOutput").ap()

    wo_r = wo_d.rearrange("(h p) o -> p h o", p=DH)       # [128, 8, 2048]

    with tile.TileContext(nc) as tc, ExitStack() as ctx:
        def pool(name, bufs, space="SBUF"):
            return ctx.enter_context(tc.tile_pool(name=name, bufs=bufs, space=space))

        xt_p = pool("xt", 6)
        wqk_p = pool("wqk", 4)
        per_p = pool("per", 1)       # persistent singles (distinct tags)
        qT_p = pool("qT", 1)
        oT_p = pool("oT", 1)
        pT_p = pool("pT", 6)
        wo_p = pool("wo", 3)
        y_p = pool("y", 2)
        rl_p = pool("rl", 1)
        rlb_p = pool("rlb", 1)
        sw_p = pool("sw", 2)
        tab_p = pool("tab", 1)
        psG = pool("psG", 3, "PSUM")
        psO = pool("psO", 4, "PSUM")
        psL = pool("psL", 1, "PSUM")

        # persistent SBUF residents
        wv_sb = per_p.tile([DH, NDC, KVL * DH], f32r, tag="wv")
        kT_sb = per_p.tile([DH, KVL, S], f32r, tag="kT")
        v_sb = per_p.tile([DH, NKT, KVL * DH], f32r, tag="v")
        msk_sb = per_p.tile([DH, 4, CH], f32r, tag="msk")
        oh_sb = per_p.tile([DH, 4, 4], f32r, tag="oh")

        def load_persistents():
            wv_r = wv_d.rearrange("(dc p) v -> p dc v", p=DH)
            nc.sync.dma_start(wv_sb[:], wv_r[:])
            nc.sync.dma_start(msk_sb[:], msk_d[:])
            nc.sync.dma_start(oh_sb[:], oh_d[:])

        def rope(tgt, cos_c, sin_c):
            """In-place RoPE on a [128, CH] slice (permuted layout:
            rotate_half = 64-partition block swap)."""
            sw = sw_p.tile([DH, CH], f32r, tag="sw", name="sw")
            nc.gpsimd.dma_start(sw[0:64, :], tgt[64:128, :])
            nc.gpsimd.dma_start(sw[64:128, :], tgt[0:64, :])
            nc.vector.tensor_mul(tgt, tgt, cos_c[:])
            nc.vector.tensor_mul(sw[:], sw[:], sin_c[:])
            nc.vector.tensor_add(tgt, tgt, sw[:])

        def load_and_project(qc):
            """x^T chunk load + Q/K projection (with inline RoPE) + V proj."""
            s0 = qc * CH
            # prefetch the first f-iteration's weight tiles ahead of the bulky
            # x^T stream so the first accumulation can start immediately
            wt_pre = []
            for dcg in range(2):
                wt = wqk_p.tile([DH, 8, DH], f32r, tag="w", name=f"wt_pre{dcg}")
                nc.sync.dma_start(wt[:], wqk_d[HL, dcg])
                wt_pre.append(wt)
            xt_g = [xt_p.tile([DH, 4, CH], f32r, tag="xt", name=f"xt_g{_i}")
                    for _i in range(4)]
            for Dc in range(NDC):
                nc.sync.dma_start(xt_g[Dc // 4][:, Dc % 4, :],
                                  xt_d[Dc * DH:(Dc + 1) * DH, s0:s0 + CH])

            def xt_slice(Dc, lo=0, hi=CH):
                return xt_g[Dc // 4][:, Dc % 4, lo:hi]

            def v_proj():
                # V PSUM drains land on an empty DVE queue (before the RoPE
                # backlog), so banks recycle fast.
                for st in range(4):
                    kt_g = qc * 4 + st
                    psv = psG.tile([DH, KVL * DH], f32, tag="g")
                    for Dc in range(NDC):
                        nc.tensor.matmul(psv[:], xt_slice(Dc, st * DH, (st + 1) * DH),
                                         wv_sb[:, Dc, :],
                                         start=(Dc == 0), stop=(Dc == NDC - 1))
                    nc.vector.tensor_copy(v_sb[:, kt_g, :], psv[:])

            if qc > 0:
                v_proj()
            cos_c = tab_p.tile([DH, CH], f32r, tag="cos", name="cos_c")
            sin_c = tab_p.tile([DH, CH], f32r, tag="sin", name="sin_c")
            nc.sync.dma_start(cos_c[:], cos_d[:, s0:s0 + CH])
            nc.sync.dma_start(sin_c[:], sin_d[:, s0:s0 + CH])
            qT_sb = qT_p.tile([DH, HL, CH], f32r, tag="qT", name="qT_sb")
            # K heads first: chunk-0 attention (all-diagonal) needs roped k^T
            # before anything else.
            for f in [HL, HL + 1] + list(range(HL)):
                ps = psG.tile([DH, CH], f32, tag="g")
                for dcg in range(2):
                    if f == HL:
                        wt = wt_pre[dcg]
                    else:
                        wt = wqk_p.tile([DH, 8, DH], f32r, tag="w")
                        nc.sync.dma_start(wt[:], wqk_d[f, dcg])
                    for dci in range(8):
                        Dc = dcg * 8 + dci
                        nc.tensor.matmul(ps[:], wt[:, dci, :], xt_slice(Dc),
                                         start=(Dc == 0), stop=(Dc == NDC - 1))
                dst = qT_sb[:, f, :] if f < HL else kT_sb[:, f - HL, s0:s0 + CH]
                nc.vector.tensor_copy(dst, ps[:])
                rope(dst, cos_c, sin_c)
                if qc == 0 and f == HL + 1:
                    load_persistents()
            if qc == 0:
                v_proj()

            return qT_sb

        qT_sb = load_and_project(0)
        for qc in range(NCH):
            s0 = qc * CH
            # ---- attention for this q-chunk ----
            oT_sb = oT_p.tile([DH, HL, CH], f32r, tag="oT")
            nkt = (qc + 1) * 4
            for g in range(2):
                po = [psO.tile([DH, CH], f32, tag="o", name=f"po{_i}") for _i in range(4)]
                pl = psL.tile([DH, CH], f32, tag="l")
                for kt in range(nkt):
                    pts = []
                    for hi in range(4):
                        h = g * 4 + hi
                        ps = psG.tile([DH, CH], f32, tag="g")
                        nc.tensor.matmul(ps[:], kT_sb[:, g, kt * DH:(kt + 1) * DH],
                                         qT_sb[:, h, :], start=True, stop=True)
                        pt = pT_p.tile([DH, CH], f32r, tag="p")
                        nc.scalar.activation(pt[:], ps[:], AF.Exp, scale=SCALE)
                        if kt >= qc * 4:
                            r = kt - qc * 4
                            nc.vector.tensor_mul(pt[:], pt[:], msk_sb[:, r, :])
                        pts.append(pt)
                    for hi in range(4):
                        nc.tensor.matmul(po[hi][:], v_sb[:, kt, g * DH:(g + 1) * DH],
                                         pts[hi][:], start=(kt == 0), stop=(kt == nkt - 1))
                    for hi in range(4):
                        nc.tensor.matmul(pl[0:4, :], oh_sb[:, hi, :], pts[hi][:],
                                         start=(kt == 0 and hi == 0),
                                         stop=(kt == nkt - 1 and hi == 3))
                # reciprocal first (frees the l-bank for the next group),
                # then drain the O accumulators (unnormalized)
                rl = rl_p.tile([4, CH], f32, tag="rl")
                nc.vector.reciprocal(rl[:], pl[0:4, :])
                for hi in range(4):
                    nc.vector.tensor_copy(oT_sb[:, g * 4 + hi, :], po[hi][:])
                for hi in range(4):
                    rlh = rl_p.tile([1, CH], f32, tag="rlh", name="rlh")
                    nc.gpsimd.dma_start(rlh[0:1, :], rl[hi:hi + 1, :])
                    rlb = rlb_p.tile([DH, CH], f32, tag="rlb")
                    nc.gpsimd.partition_broadcast(rlb[:], rlh[0:1, :])
                    nc.vector.tensor_mul(oT_sb[:, g * 4 + hi, :],
                                         oT_sb[:, g * 4 + hi, :], rlb[:])

            # ---- next chunk's projection (fills PE while the softmax
            # normalization round-trip completes) ----
            if qc + 1 < NCH:
                qT_next = load_and_project(qc + 1)
            else:
                qT_next = None

            # ---- output projection ----
            for oc in range(4):
                wta = wo_p.tile([DH, 4, CH], f32r, tag="wo", name="wta")
                wtb = wo_p.tile([DH, 4, CH], f32r, tag="wo", name="wtb")
                nc.sync.dma_start(wta[:], wo_r[:, 0:4, oc * CH:(oc + 1) * CH])
                nc.sync.dma_start(wtb[:], wo_r[:, 4:8, oc * CH:(oc + 1) * CH])
                for qt in range(4):
                    psy = psG.tile([DH, CH], f32, tag="g")
                    for hid in range(HL):
                        wt_h = wta[:, hid, :] if hid < 4 else wtb[:, hid - 4, :]
                        nc.tensor.matmul(psy[:], oT_sb[:, hid, qt * DH:(qt + 1) * DH],
                                         wt_h, start=(hid == 0), stop=(hid == HL - 1))
                    yt = y_p.tile([DH, CH], f32, tag="y")
                    nc.vector.tensor_copy(yt[:], psy[:])
                    nc.sync.dma_start(
                        y_d[s0 + qt * DH: s0 + (qt + 1) * DH, oc * CH:(oc + 1) * CH], yt[:])
            qT_sb = qT_next

    nc.compile()
    return nc


def _get_nc():
    global _NC_CACHE
    if _NC_CACHE is None:
        _NC_CACHE = build_nc()
    return _NC_CACHE


def _host_prep(x, Wqkv, Wo):
    perm = np.concatenate([np.arange(0, DH, 2), np.arange(1, DH, 2)])
    # RoPE tables, matching reference numerics (float32 throughout)
    theta = (1.0 / (np.float32(10000.0) **
                    (np.arange(0, DH, 2, dtype=np.float32) / np.float32(DH))))
    theta = np.repeat(theta.astype(np.float32), 2)                  # [128]
    ang = (np.arange(S, dtype=np.float32)[:, None] * theta[None, :])  # [S, 128]
    cosT = np.ascontiguousarray(np.cos(ang).astype(np.float32).T[perm, :])  # [128, S]
    sinT = np.ascontiguousarray(np.sin(ang).astype(np.float32).T[perm, :])
    sinT[:64, :] *= np.float32(-1.0)

    kk = np.arange(DH)[:, None]
    qq = np.arange(CH)[None, :]
    dmask = np.zeros((DH, 4, CH), np.float32)
    for r in range(4):
        dmask[:, r, :] = (qq >= r * DH + kk).astype(np.float32)
    oh = np.zeros((DH, 4, 4), np.float32)
    for hi in range(4):
        oh[:, hi, hi] = 1.0

    in_maps = []
    for c in range(8):
        b, r = c // 2, c % 2
        xt = np.ascontiguousarray(x[b].T)
        cols = []
        for h in range(HL):
            base = (r * HL + h) * DH
            cols.append(Wqkv[:, base + perm])
        for l in range(KVL):
            base = HQ * DH + (r * KVL + l) * DH
            cols.append(Wqkv[:, base + perm])
        wqk = np.concatenate(cols, axis=1)              # [2048, 1280]
        # pre-tile: [f, dcg, p, dc, fi] with 4KB-contiguous per-partition lines
        wqk = np.ascontiguousarray(
            wqk.reshape(2, 8, DH, HL + KVL, DH)          # [dcg, dc, p, f, fi]
               .transpose(3, 0, 2, 1, 4))                # [f, dcg, p, dc, fi]
        vbase = (HQ + HKV) * DH
        wv = np.ascontiguousarray(Wqkv[:, vbase + r * KVL * DH: vbase + (r + 1) * KVL * DH])
        wo = np.ascontiguousarray(Wo[r * HL * DH:(r + 1) * HL * DH, :])
        in_maps.append(dict(xt=xt, wqk=wqk, wv=wv, wo=wo, cost=cosT, sint=sinT,
                            dmask=dmask, onehot=oh))
    return in_maps


def kernel(x, Wqkv, Wo, mask):
    global LAST_EXEC_NS
    x = np.asarray(x, dtype=np.float32)
    Wqkv = np.asarray(Wqkv, dtype=np.float32)
    Wo = np.asarray(Wo, dtype=np.float32)
    in_maps = _host_prep(x, Wqkv, Wo)
    nc = _get_nc()
    trace = os.environ.get("KTRACE", "0") == "1"
    res = bass_utils.run_bass_kernel_spmd(nc, in_maps, core_ids=list(range(8)), trace=trace)
    LAST_EXEC_NS = res.exec_time_ns
    out = np.empty((B, S, D), np.float32)
    for b in range(B):
        out[b] = res.results[2 * b]["y"] + res.results[2 * b + 1]["y"]
    return out


# revision 29
# speedup vs baseline: 1.0172x; 1.0004x over previous
"""Trainium2 Bass kernel for nn_MultiHeadSelfAttention_24541443130010.

Full inputs in, full output out. Sharding: DP over batch (4) x TP over heads (2)
across 8 NeuronCores. Core c handles batch c//2 with q-heads [8r, 8r+8) and
kv-heads [2r, 2r+2), r = c%2. Wo is row-sliced per rank; the TP partial sums
are reduced on the host after gather.

Device kernel (per core, fp32 storage / float32r matmuls):
  qkv^T projection from host-pretransposed x^T, RoPE with the head-dim
  de-interleave permutation folded into the Wq/Wk columns (rotate_half becomes
  a 64-partition block swap), flash-style attention computing S^T = K Q^T tiles
  (softmax denominators via one-hot ones-matmuls on PE; no max subtraction --
  scores are O(1) here), causal block skipping, then the output projection.
"""
import os
import numpy as np
from contextlib import ExitStack

import concourse.bass as bass
import concourse.mybir as mybir
import concourse.tile as tile
from concourse import bass_utils, bacc

f32 = mybir.dt.float32
f32r = mybir.dt.float32r
AF = mybir.ActivationFunctionType

B = 4
S = 2048
D = 2048
DH = 128
HQ = 16
HKV = 4
TP = 2
HL = HQ // TP       # 8 local q heads
KVL = HKV // TP     # 2 local kv heads
CH = 512            # q-chunk size
NCH = S // CH       # 4
NKT = S // DH       # 16 k-tiles
NDC = D // DH       # 16 contraction chunks
SCALE = 1.0 / float(np.sqrt(DH))

LAST_EXEC_NS = None
_NC_CACHE = None


def build_nc():
    nc = bacc.Bacc("TRN2", target_bir_lowering=False, debug=False)
    xt_d = nc.dram_tensor("xt", [D, S], f32r, kind="ExternalInput").ap()
    wqk_d = nc.dram_tensor("wqk", [HL + KVL, 2, DH, 8, DH], f32r, kind="ExternalInput").ap()
    wv_d = nc.dram_tensor("wv", [D, KVL * DH], f32r, kind="ExternalInput").ap()
    wo_d = nc.dram_tensor("wo", [HL * DH, D], f32r, kind="ExternalInput").ap()
    cos_d = nc.dram_tensor("cost", [DH, S], f32r, kind="ExternalInput").ap()
    sin_d = nc.dram_tensor("sint", [DH, S], f32r, kind="ExternalInput").ap()
    msk_d = nc.dram_tensor("dmask", [DH, 4, CH], f32r, kind="ExternalInput").ap()
    oh_d = nc.dram_tensor("onehot", [DH, 4, 4], f32r, kind="ExternalInput").ap()
    y_d = nc.dram_tensor("y", [S, D], f32, kind="ExternalOutput").ap()

    wo_r = wo_d.rearrange("(h p) o -> p h o", p=DH)       # [128, 8, 2048]

    with tile.TileContext(nc) as tc, ExitStack() as ctx:
        def pool(name, bufs, space="SBUF"):
            return ctx.enter_context(tc.tile_pool(name=name, bufs=bufs, space=space))

        xt_p = pool("xt", 6)
        wqk_p = pool("wqk", 4)
        per_p = pool("per", 1)       # persistent singles (distinct tags)
        qT_p = pool("qT", 1)
        oT_p = pool("oT", 1)
        pT_p = pool("pT", 6)
        wo_p = pool("wo", 3)
        y_p = pool("y", 2)
        rl_p = pool("rl", 1)
        rlb_p = pool("rlb", 1)
        sw_p = pool("sw", 2)
        tab_p = pool("tab", 1)
        psG = pool("psG", 3, "PSUM")
        psO = pool("psO", 4, "PSUM")
        psL = pool("psL", 1, "PSUM")

        # persistent SBUF residents
        wv_sb = per_p.tile([DH, NDC, KVL * DH], f32r, tag="wv")
        kT_sb = per_p.tile([DH, KVL, S], f32r, tag="kT")
        v_sb = per_p.tile([DH, NKT, KVL * DH], f32r, tag="v")
        msk_sb = per_p.tile([DH, 4, CH], f32r, tag="msk")
        oh_sb = per_p.tile([DH, 4, 4], f32r, tag="oh")

        def load_persistents():
            wv_r = wv_d.rearrange("(dc p) v -> p dc v", p=DH)
            nc.sync.dma_start(wv_sb[:], wv_r[:])
            nc.sync.dma_start(msk_sb[:], msk_d[:])
            nc.sync.dma_start(oh_sb[:], oh_d[:])

        def rope(tgt, cos_c, sin_c):
            """In-place RoPE on a [128, CH] slice (permuted layout:
            rotate_half = 64-partition block swap)."""
            sw = sw_p.tile([DH, CH], f32r, tag="sw", name="sw")
            nc.gpsimd.dma_start(sw[0:64, :], tgt[64:128, :])
            nc.gpsimd.dma_start(sw[64:128, :], tgt[0:64, :])
            nc.vector.tensor_mul(tgt, tgt, cos_c[:])
            nc.vector.tensor_mul(sw[:], sw[:], sin_c[:])
            nc.vector.tensor_add(tgt, tgt, sw[:])

        def load_and_project(qc):
            """x^T chunk load + Q/K projection (with inline RoPE) + V proj."""
            s0 = qc * CH
            # prefetch the first f-iteration's weight tiles ahead of the bulky
            # x^T stream so the first accumulation can start immediately
            wt_pre = []
            for dcg in range(2):
                wt = wqk_p.tile([DH, 8, DH], f32r, tag="w", name=f"wt_pre{dcg}")
                nc.sync.dma_start(wt[:], wqk_d[HL, dcg])
                wt_pre.append(wt)
            xt_g = [xt_p.tile([DH, 4, CH], f32r, tag="xt", name=f"xt_g{_i}")
                    for _i in range(4)]
            for Dc in range(NDC):
                nc.sync.dma_start(xt_g[Dc // 4][:, Dc % 4, :],
                                  xt_d[Dc * DH:(Dc + 1) * DH, s0:s0 + CH])

            def xt_slice(Dc, lo=0, hi=CH):
                return xt_g[Dc // 4][:, Dc % 4, lo:hi]

            def v_proj():
                # V PSUM drains land on an empty DVE queue (before the RoPE
                # backlog), so banks recycle fast.
                for st in range(4):
                    kt_g = qc * 4 + st
                    psv = psG.tile([DH, KVL * DH], f32, tag="g")
                    for Dc in range(NDC):
                        nc.tensor.matmul(psv[:], xt_slice(Dc, st * DH, (st + 1) * DH),
                                         wv_sb[:, Dc, :],
                                         start=(Dc == 0), stop=(Dc == NDC - 1))
                    nc.vector.tensor_copy(v_sb[:, kt_g, :], psv[:])

            if qc > 0:
                v_proj()
            cos_c = tab_p.tile([DH, CH], f32r, tag="cos", name="cos_c")
            sin_c = tab_p.tile([DH, CH], f32r, tag="sin", name="sin_c")
            nc.sync.dma_start(cos_c[:], cos_d[:, s0:s0 + CH])
            nc.sync.dma_start(sin_c[:], sin_d[:, s0:s0 + CH])
            qT_sb = qT_p.tile([DH, HL, CH], f32r, tag="qT", name="qT_sb")
            # K heads first: chunk-0 attention (all-diagonal) needs roped k^T
            # before anything else.
            for f in [HL, HL + 1] + list(range(HL)):
                ps = psG.tile([DH, CH], f32, tag="g")
                for dcg in range(2):
                    if f == HL:
                        wt = wt_pre[dcg]
                    else:
                        wt = wqk_p.tile([DH, 8, DH], f32r, tag="w")
                        nc.sync.dma_start(wt[:], wqk_d[f, dcg])
                    for dci in range(8):
                        Dc = dcg * 8 + dci
                        nc.tensor.matmul(ps[:], wt[:, dci, :], xt_slice(Dc),
                                         start=(Dc == 0), stop=(Dc == NDC - 1))
                dst = qT_sb[:, f, :] if f < HL else kT_sb[:, f - HL, s0:s0 + CH]
                nc.vector.tensor_copy(dst, ps[:])
                rope(dst, cos_c, sin_c)
                if qc == 0 and f == HL + 1:
                    load_persistents()
            if qc == 0:
                v_proj()

            return qT_sb

        qT_sb = load_and_project(0)
        for qc in range(NCH):
            s0 = qc * CH
            # ---- attention for this q-chunk ----
            oT_sb = oT_p.tile([DH, HL, CH], f32r, tag="oT")
            nkt = (qc + 1) * 4
            for g in range(2):
                po = [psO.tile([DH, CH], f32, tag="o", name=f"po{_i}") for _i in range(4)]
                pl = psL.tile([DH, CH], f32, tag="l")
                for kt in range(nkt):
                    pts = []
                    for hi in range(4):
                        h = g * 4 + hi
                        ps = psG.tile([DH, CH], f32, tag="g")
                        nc.tensor.matmul(ps[:], kT_sb[:, g, kt * DH:(kt + 1) * DH],
                                         qT_sb[:, h, :], start=True, stop=True)
                        pt = pT_p.tile([DH, CH], f32r, tag="p")
                        nc.scalar.activation(pt[:], ps[:], AF.Exp, scale=SCALE)
                        if kt >= qc * 4:
                            r = kt - qc * 4
                            nc.vector.tensor_mul(pt[:], pt[:], msk_sb[:, r, :])
                        pts.append(pt)
                    for hi in range(4):
                        nc.tensor.matmul(po[hi][:], v_sb[:, kt, g * DH:(g + 1) * DH],
                                         pts[hi][:], start=(kt == 0), stop=(kt == nkt - 1))
                        nc.tensor.matmul(pl[0:4, :], oh_sb[:, hi, :], pts[hi][:],
                                         start=(kt == 0 and hi == 0),
                                         stop=(kt == nkt - 1 and hi == 3))
                # reciprocal first (frees the l-bank for the next group),
                # then drain the O accumulators (unnormalized)
                rl = rl_p.tile([4, CH], f32, tag="rl")
                nc.vector.reciprocal(rl[:], pl[0:4, :])
                for hi in range(4):
                    nc.vector.tensor_copy(oT_sb[:, g * 4 + hi, :], po[hi][:])
                for hi in range(4):
                    rlh = rl_p.tile([1, CH], f32, tag="rlh", name="rlh")
                    nc.gpsimd.dma_start(rlh[0:1, :], rl[hi:hi + 1, :])
                    rlb = rlb_p.tile([DH, CH], f32, tag="rlb")
                    nc.gpsimd.partition_broadcast(rlb[:], rlh[0:1, :])
                    nc.vector.tensor_mul(oT_sb[:, g * 4 + hi, :],
                                         oT_sb[:, g * 4 + hi, :], rlb[:])

            # ---- next chunk's projection (fills PE while the softmax
            # normalization round-trip completes) ----
            if qc + 1 < NCH:
                qT_next = load_and_project(qc + 1)
            else:
                qT_next = None

            # ---- output projection ----
            for oc in range(4):
                wta = wo_p.tile([DH, 4, CH], f32r, tag="wo", name="wta")
                wtb = wo_p.tile([DH, 4, CH], f32r, tag="wo", name="wtb")
                nc.sync.dma_start(wta[:], wo_r[:, 0:4, oc * CH:(oc + 1) * CH])
                nc.sync.dma_start(wtb[:], wo_r[:, 4:8, oc * CH:(oc + 1) * CH])
                for qt in range(4):
                    psy = psG.tile([DH, CH], f32, tag="g")
                    for hid in range(HL):
                        wt_h = wta[:, hid, :] if hid < 4 else wtb[:, hid - 4, :]
                        nc.tensor.matmul(psy[:], oT_sb[:, hid, qt * DH:(qt + 1) * DH],
                                         wt_h, start=(hid == 0), stop=(hid == HL - 1))
                    yt = y_p.tile([DH, CH], f32, tag="y")
                    nc.vector.tensor_copy(yt[:], psy[:])
                    nc.sync.dma_start(
                        y_d[s0 + qt * DH: s0 + (qt + 1) * DH, oc * CH:(oc + 1) * CH], yt[:])
            qT_sb = qT_next

    nc.compile()
    return nc


def _get_nc():
    global _NC_CACHE
    if _NC_CACHE is None:
        _NC_CACHE = build_nc()
    return _NC_CACHE


def _host_prep(x, Wqkv, Wo):
    perm = np.concatenate([np.arange(0, DH, 2), np.arange(1, DH, 2)])
    # RoPE tables, matching reference numerics (float32 throughout)
    theta = (1.0 / (np.float32(10000.0) **
                    (np.arange(0, DH, 2, dtype=np.float32) / np.float32(DH))))
    theta = np.repeat(theta.astype(np.float32), 2)                  # [128]
    ang = (np.arange(S, dtype=np.float32)[:, None] * theta[None, :])  # [S, 128]
    cosT = np.ascontiguousarray(np.cos(ang).astype(np.float32).T[perm, :])  # [128, S]
    sinT = np.ascontiguousarray(np.sin(ang).astype(np.float32).T[perm, :])
    sinT[:64, :] *= np.float32(-1.0)

    kk = np.arange(DH)[:, None]
    qq = np.arange(CH)[None, :]
    dmask = np.zeros((DH, 4, CH), np.float32)
    for r in range(4):
        dmask[:, r, :] = (qq >= r * DH + kk).astype(np.float32)
    oh = np.zeros((DH, 4, 4), np.float32)
    for hi in range(4):
        oh[:, hi, hi] = 1.0

    in_maps = []
    for c in range(8):
        b, r = c // 2, c % 2
        xt = np.ascontiguousarray(x[b].T)
        cols = []
        for h in range(HL):
            base = (r * HL + h) * DH
            cols.append(Wqkv[:, base + perm])
        for l in range(KVL):
            base = HQ * DH + (r * KVL + l) * DH
            cols.append(Wqkv[:, base + perm])
        wqk = np.concatenate(cols, axis=1)              # [2048, 1280]
        # pre-tile: [f, dcg, p, dc, fi] with 4KB-contiguous per-partition lines
        wqk = np.ascontiguousarray(
            wqk.reshape(2, 8, DH, HL + KVL, DH)          # [dcg, dc, p, f, fi]
               .transpose(3, 0, 2, 1, 4))                # [f, dcg, p, dc, fi]
        vbase = (HQ + HKV) * DH
        wv = np.ascontiguousarray(Wqkv[:, vbase + r * KVL * DH: vbase + (r + 1) * KVL * DH])
        wo = np.ascontiguousarray(Wo[r * HL * DH:(r + 1) * HL * DH, :])
        in_maps.append(dict(xt=xt, wqk=wqk, wv=wv, wo=wo, cost=cosT, sint=sinT,
                            dmask=dmask, onehot=oh))
    return in_maps


def kernel(x, Wqkv, Wo, mask):
    global LAST_EXEC_NS
    x = np.asarray(x, dtype=np.float32)
    Wqkv = np.asarray(Wqkv, dtype=np.float32)
    Wo = np.asarray(Wo, dtype=np.float32)
    in_maps = _host_prep(x, Wqkv, Wo)
    nc = _get_nc()
    trace = os.environ.get("KTRACE", "0") == "1"
    res = bass_utils.run_bass_kernel_spmd(nc, in_maps, core_ids=list(range(8)), trace=trace)
    LAST_EXEC_NS = res.exec_time_ns
    out = np.empty((B, S, D), np.float32)
    for b in range(B):
        out[b] = res.results[2 * b]["y"] + res.results[2 * b + 1]["y"]
    return out
